# revision 33
# baseline (speedup 1.0000x reference)
"""Trainium2 Bass kernel for nn_ASPECTS_multiloss (focal multi-loss over [2M, 20]).

Strategy: pure data-parallel over 8 NeuronCores (250k rows each). Host converts
x, y to fp16 (halves DMA bytes; DVE tensor_tensor then runs in 2x packed mode).

Math (ALPHA=1, GAMMA=2):
  bce  = softplus(x) - x*y
  term = y * (1 - exp(-bce))^2 * bce          -> focal = mean over B*20
  y_sum = sum_i y[b,i,j]; x_mean = mean_i x; x_min = min_i x   (i in 0..9)
  aspect: focal(x_mean*hs_w + hs_b, [y_sum >= 6])  -> mean over B*2
  detect: focal(x_min, [y_sum >= 10])              -> mean over B*2
  cs_loss == 0 exactly (relu(-x) * relu(min_i x) has one factor == 0 per elem)
  out = focal + aspect + 0.5*detect

The compiler's ACT tables have no softplus, so softplus(x) = Ln(Exp(x) + 1)
(|x| <= ~7 for randn inputs; fp16 exp overflows only past 11.09). All ACT
functions used (Exp, Ln, Square, Identity) live in the single
natural_log_exp_and_others table set -> exactly one ACT_TABLE_LOAD.

Engine plan per tile [128 part x 128 rows x 20 cols]:
  ACT: e=Exp(x); s=Ln(e+1); pt=Exp(-b); q=Square(pt-1)
  DVE: u=x*y; b=s-u; by=b*y; w=q*by; pairwise group-stat trees
       (y-tree in f32: fp16 y_sum quantizes at the 6.0 threshold and biases
        the aspect loss ~4e-3; x-trees stay fp16)
  PE:  final sums via ones-matmul accumulated in PSUM (f32, exact), since
       tensor_tensor_reduce faults at runtime on this stack.
  Small chains run batched over SBUF-staged stats. Host combines partials.

x layout note: row cols 0:10 == (i in 0:5, j), cols 10:20 == (i in 5:10, j),
so the tree's level-1 operands are contiguous half-row slices.
"""

import numpy as np
from contextlib import ExitStack

import concourse.bass as bass
import concourse.bacc as bacc
import concourse.tile as tile
import concourse.mybir as mybir
from concourse.bass_utils import run_bass_kernel_spmd

AF = mybir.ActivationFunctionType
ALU = mybir.AluOpType
FP16 = mybir.dt.float16
F32 = mybir.dt.float32

N_CORES = 8
B_TOTAL = 2_000_000
ROWS = B_TOTAL // N_CORES          # 250_000 rows per core
P = 128                            # partitions
G = 128                            # row-groups per partition per full tile
TILE_ROWS = P * G                  # 16384
T_FULL = ROWS // TILE_ROWS         # 15 full tiles at G=128
TAIL_ROWS = ROWS - T_FULL * TILE_ROWS   # 4240
TAIL_P, TAIL_G = 106, 40           # 106*40 == 4240
N_TILES = T_FULL + 1
STAGE_W = T_FULL * G * 2 + TAIL_G * 2   # 3920 staging columns
SMALL_N = 4                        # small-chain column chunks
SMALL_W = STAGE_W // SMALL_N       # 980

ASPECT_TH = 6.0
DETECT_TH = 10.0

PS_F, PS_S = 512, 490              # psum widths: focal chunk, small chunk
OUT_W = PS_F + 2 * PS_S            # [1, 1492] output: focal | aspect | detect


def _grp(ap, g, i, j=2):
    return ap.rearrange("p (g i j) -> p g i j", g=g, i=i, j=j)


def _tree(nc, pool, p, g, in_a3, in_b3, out2, op, mid_dt, eng=None, tag="tree",
          l1_dt=None):
    """Reduce 10 group values (two [p, g, 10]-contiguous halves, i.e. (i in
    0:5, j) and (i in 5:10, j)) to [p, g, 2] (out2). All operands are 3-d
    APs with contiguous innermost runs (the (i, j) pairs merge), which every
    engine's codegen supports and which keeps DVE 2x packing eligible."""
    eng = eng or nc.vector
    l1 = pool.tile([p, g * 10], l1_dt or mid_dt, tag=f"{tag}_l1")
    l1v = l1.rearrange("p (g c) -> p g c", g=g, c=10)
    eng.tensor_tensor(l1v, in_a3, in_b3, op=op)
    l2 = pool.tile([p, g * 4], mid_dt, tag=f"{tag}_l2")
    l2v = l2.rearrange("p (g c) -> p g c", g=g, c=4)
    eng.tensor_tensor(l2v, l1v[:, :, 0:4], l1v[:, :, 4:8], op=op)
    l3 = pool.tile([p, g * 2], mid_dt, tag=f"{tag}_l3")
    l3v = l3.rearrange("p (g c) -> p g c", g=g, c=2)
    eng.tensor_tensor(l3v, l2v[:, :, 0:2], l2v[:, :, 2:4], op=op)
    eng.tensor_tensor(out2, l3v, l1v[:, :, 8:10], op=op)


def build_bass():
    nc = bacc.Bacc("TRN2", target_bir_lowering=False, num_devices=N_CORES)

    x_in = nc.declare_dram_parameter("x_in", [ROWS, 20], FP16, isOutput=False)
    y_in = nc.declare_dram_parameter("y_in", [ROWS, 20], FP16, isOutput=False)
    w10 = nc.declare_dram_parameter("w10", [P, 1], F32, isOutput=False)
    hbp = nc.declare_dram_parameter("hbp", [P, 1], F32, isOutput=False)
    out = nc.declare_dram_parameter("out", [1, OUT_W], F32, isOutput=True)

    main_rows = T_FULL * TILE_ROWS

    def main_view(t):
        return t[:][0:main_rows, :].rearrange(
            "(t p g) c -> t p (g c)", t=T_FULL, p=P, g=G
        )

    def tail_view(t):
        return t[:][main_rows:ROWS, :].rearrange(
            "(p g) c -> p (g c)", p=TAIL_P, g=TAIL_G
        )

    x_m, y_m = main_view(x_in), main_view(y_in)
    x_t, y_t = tail_view(x_in), tail_view(y_in)

    with ExitStack() as ctx:
        tc = ctx.enter_context(tile.TileContext(nc))
        io = ctx.enter_context(tc.tile_pool(name="io", bufs=3))
        work = ctx.enter_context(tc.tile_pool(name="work", bufs=2))
        persist = ctx.enter_context(tc.tile_pool(name="persist", bufs=1))
        small = ctx.enter_context(tc.tile_pool(name="small", bufs=1))
        psum = ctx.enter_context(tc.tile_pool(name="psum", bufs=1, space="PSUM"))

        # --- persistent state
        ysum_st = persist.tile([P, STAGE_W], F32, tag="ysum_st")
        xsum_st = persist.tile([P, STAGE_W], FP16, tag="xsum_st")
        xmin_st = persist.tile([P, STAGE_W], FP16, tag="xmin_st")
        if TAIL_P < P:
            # only the tail tile's unused partitions are never written
            c0 = T_FULL * G * 2
            p0 = (TAIL_P // 32) * 32  # partition starts must be 32-aligned;
            for st in (ysum_st, xsum_st, xmin_st):
                # rows p0:TAIL_P are re-written by the tail tile afterwards
                nc.vector.memset(st[p0:P, c0:STAGE_W], 0.0)
        w10_t = persist.tile([P, 1], F32, tag="w10_t")
        nc.sync.dma_start(w10_t, w10[:])
        hb_t = persist.tile([P, 1], F32, tag="hb_t")
        nc.sync.dma_start(hb_t, hbp[:])
        bias_m1 = persist.tile([P, 1], F32, tag="bias_m1")
        nc.vector.memset(bias_m1, -1.0)
        ones = persist.tile([P, 1], FP16, tag="ones")
        nc.vector.memset(ones, 1.0)

        ps_f = psum.tile([1, PS_F], F32, tag="ps_f")
        ps_a = psum.tile([1, PS_S], F32, tag="ps_a")
        ps_d = psum.tile([1, PS_S], F32, tag="ps_d")

        def tile_params(ti):
            if ti < T_FULL:
                return P, G, x_m[ti], y_m[ti]
            return TAIL_P, TAIL_G, x_t, y_t

        def small_chunk(si):
            """Aspect+detect chains over staged-stat columns [si*SMALL_W ...)."""
            s0 = si * SMALL_W
            ys = ysum_st[:, s0 : s0 + SMALL_W]
            for which, ps in (("aspect", ps_a), ("detect", ps_d)):
                yth = small.tile([P, SMALL_W], FP16, tag="sm_yth")
                if which == "aspect":
                    xv = small.tile([P, SMALL_W], FP16, tag="sm_xhs")
                    nc.vector.tensor_scalar(
                        xv, xsum_st[:, s0 : s0 + SMALL_W], w10_t, hb_t,
                        op0=ALU.mult, op1=ALU.add,
                    )
                    nc.vector.tensor_scalar(yth, ys, ASPECT_TH, None, op0=ALU.is_ge)
                else:
                    xv = xmin_st[:, s0 : s0 + SMALL_W]
                    nc.vector.tensor_scalar(yth, ys, DETECT_TH, None, op0=ALU.is_ge)

                e2 = small.tile([P, SMALL_W], F32, tag="sm_e")
                nc.scalar.activation(e2, xv, AF.Exp)
                s2 = small.tile([P, SMALL_W], FP16, tag="sm_s")
                nc.scalar.activation(s2, e2, AF.Ln, bias=1.0)
                u2 = small.tile([P, SMALL_W], FP16, tag="sm_u")
                nc.vector.tensor_tensor(u2, xv, yth, op=ALU.mult)
                b2 = small.tile([P, SMALL_W], FP16, tag="sm_b")
                nc.vector.tensor_tensor(b2, s2, u2, op=ALU.subtract)
                pt2 = small.tile([P, SMALL_W], FP16, tag="sm_pt")
                nc.scalar.activation(pt2, b2, AF.Exp, scale=-1.0)
                q2 = small.tile([P, SMALL_W], FP16, tag="sm_q")
                nc.scalar.activation(q2, pt2, AF.Square, bias=bias_m1)
                by2 = small.tile([P, SMALL_W], FP16, tag="sm_by")
                nc.vector.tensor_tensor(by2, b2, yth, op=ALU.mult)
                w2t = small.tile([P, SMALL_W], FP16, tag="sm_u")
                nc.vector.tensor_tensor(w2t, q2, by2, op=ALU.mult)
                wv = w2t.rearrange("p (c n) -> p c n", c=2, n=PS_S)
                for c in range(2):
                    nc.tensor.matmul(
                        ps, lhsT=ones, rhs=wv[:, c, :],
                        start=(si == 0 and c == 0),
                        stop=(si == SMALL_N - 1 and c == 1),
                    )

        next_small = [0]
        for ti in range(N_TILES):
            p, g, vx, vy = tile_params(ti)
            F = g * 20
            half = F // 2
            xt = io.tile([p, F], FP16, tag="xt")
            nc.sync.dma_start(xt, vx)
            yt = io.tile([p, F], FP16, tag="yt")
            nc.sync.dma_start(yt, vy)

            # softplus(x) = Ln(Exp(x) + 1)
            e = work.tile([p, F], FP16, tag="e")
            nc.scalar.activation(e, xt, AF.Exp)
            s = work.tile([p, F], FP16, tag="s")
            nc.scalar.activation(s, e, AF.Ln, bias=1.0)
            u = work.tile([p, F], FP16, tag="u")
            nc.vector.tensor_tensor(u, xt, yt, op=ALU.mult)
            b = work.tile([p, F], FP16, tag="b")
            nc.vector.tensor_tensor(b, s, u, op=ALU.subtract)
            pt = work.tile([p, F], FP16, tag="pt")
            nc.scalar.activation(pt, b, AF.Exp, scale=-1.0)
            q = work.tile([p, F], FP16, tag="q")
            nc.scalar.activation(q, pt, AF.Square, bias=bias_m1[0:p])
            by = work.tile([p, F], FP16, tag="by")
            nc.vector.tensor_tensor(by, b, yt, op=ALU.mult)
            w = work.tile([p, F], FP16, tag="w")
            nc.vector.tensor_tensor(w, q, by, op=ALU.mult)

            # focal partial sums: PSUM += ones.T @ w (per 512-col chunk)
            n_chunks = F // PS_F if F % PS_F == 0 else None
            if n_chunks:
                wv = w.rearrange("p (c n) -> p c n", c=n_chunks, n=PS_F)
                for c in range(n_chunks):
                    nc.tensor.matmul(
                        ps_f, lhsT=ones[0:p], rhs=wv[:, c, :],
                        start=(ti == 0 and c == 0), stop=False,
                    )
            else:  # tail: 800 = 2 x 400
                wv = w.rearrange("p (c n) -> p c n", c=2, n=400)
                for c in range(2):
                    nc.tensor.matmul(
                        ps_f[:, 0:400], lhsT=ones[0:p], rhs=wv[:, c, :],
                        start=False, stop=(c == 1),
                    )

            # group stats into staging columns [ti*G*2 ...)
            x20 = xt.rearrange("p (g c) -> p g c", g=g, c=20)
            y20 = yt.rearrange("p (g c) -> p g c", g=g, c=20)
            x4a, x4b = x20[:, :, 0:10], x20[:, :, 10:20]
            y4a, y4b = y20[:, :, 0:10], y20[:, :, 10:20]
            col0 = ti * G * 2
            w2 = g * 2

            def stage(st):
                return st[0:p, col0 : col0 + w2].rearrange("p (g j) -> p g j", g=g, j=2)

            _tree(nc, work, p, g, y4a, y4b, stage(ysum_st), ALU.add, F32,
                  l1_dt=FP16)
            _tree(nc, work, p, g, x4a, x4b, stage(xsum_st), ALU.add, FP16)
            _tree(nc, work, p, g, x4a, x4b, stage(xmin_st), ALU.min, FP16,
                  tag="mtree")

            # small-chain chunk si reads stage columns written by earlier
            # tiles, so it can interleave with the main tile loop once
            # (ti+1) tiles have staged enough columns
            while next_small[0] < SMALL_N and (
                (ti + 1) * G * 2 >= (next_small[0] + 1) * SMALL_W or ti == N_TILES - 1
            ):
                small_chunk(next_small[0])
                next_small[0] += 1

        # evacuate PSUM -> SBUF -> DRAM
        sb = persist.tile([1, OUT_W], F32, tag="sb")
        nc.scalar.copy(sb[:, 0:PS_F], ps_f)
        nc.scalar.copy(sb[:, PS_F : PS_F + PS_S], ps_a)
        nc.scalar.copy(sb[:, PS_F + PS_S : OUT_W], ps_d)
        nc.sync.dma_start(out[:], sb)

    # Full bacc lowering (wait splitting, reg alloc, nop fusion, act table
    # loads) — the finalization bass_test_utils.run_kernel applies before
    # handing a Tile kernel to run_bass_kernel_spmd.
    #
    # The act-table chooser takes the first set containing each function,
    # which ping-pongs exp_and_others <-> natural_log per tile (~49 table
    # loads, ~63us). Hide the shared functions from every other set so all
    # activations resolve to natural_log_exp_and_others (indices preserved).
    import concourse.hw_specs as hw_specs

    keep = "natural_log_exp_and_others"
    shared = {AF.Exp, AF.Ln, AF.Square, AF.Identity, AF.Copy, AF.Relu, AF.Abs}
    real_tables = hw_specs.get_activation_tables(nc.m.arch)
    assert keep in real_tables and shared - {AF.Copy} <= real_tables[keep] | {AF.Copy}

    def _forced_tables(arch):
        tabs = hw_specs.get_activation_tables(arch)
        return {n: (f if n == keep else f - shared) for n, f in tabs.items()}

    orig = bacc.get_activation_tables
    bacc.get_activation_tables = _forced_tables
    try:
        nc.compile()
    finally:
        bacc.get_activation_tables = orig
    return nc


_NC_CACHE = None


def _get_nc():
    global _NC_CACHE
    if _NC_CACHE is None:
        _NC_CACHE = build_bass()
    return _NC_CACHE


def make_in_maps(x, y, hs_w, hs_b):
    w10v = np.float32(np.asarray(hs_w).reshape(-1)[0]) * np.float32(0.1)
    hbv = np.float32(np.asarray(hs_b).reshape(-1)[0])
    w10 = np.full((P, 1), w10v, np.float32)
    hbp = np.full((P, 1), hbv, np.float32)
    in_maps = []
    for c in range(N_CORES):
        in_maps.append(
            {
                "x_in": np.ascontiguousarray(x[c * ROWS : (c + 1) * ROWS], np.float16),
                "y_in": np.ascontiguousarray(y[c * ROWS : (c + 1) * ROWS], np.float16),
                "w10": w10,
                "hbp": hbp,
            }
        )
    return in_maps


def combine(results):
    Sf = Sa = Sd = 0.0
    for r in results:
        o = np.asarray(r["out"]).astype(np.float64)[0]
        Sf += o[0:PS_F].sum()
        Sa += o[PS_F : PS_F + PS_S].sum()
        Sd += o[PS_F + PS_S : OUT_W].sum()
    n_main = float(B_TOTAL * 20)
    n_small = float(B_TOTAL * 2)
    return np.float32(Sf / n_main + Sa / n_small + 0.5 * (Sd / n_small))


def kernel(x, y, hs_w, hs_b):
    x = np.asarray(x)
    y = np.asarray(y)
    nc = _get_nc()
    in_maps = make_in_maps(x, y, hs_w, hs_b)
    res = run_bass_kernel_spmd(nc, in_maps, list(range(N_CORES))).results
    return combine(res)
